# revision 19
# baseline (speedup 1.0000x reference)
"""Embedding lookup (nn.Embedding) on 8 Trainium2 NeuronCores.

Strategy: data-parallel shard token_ids along the batch dim (8 batch rows ->
8 cores); cast the table to bf16 on the host, write the OUTPUT as bf16 and
upcast to f32 on the host (harness gate is rel_err < 2e-2; bf16 keeps
per-element error <= 2^-8 ~= 0.4%). On-device bytes per core: 8 MB gather
read + 8 MB output write vs the ~358 GB/s HBM-per-NeuronCore cap -> ~46 us
data floor.

Fast path (gather module): the f32->bf16 indirect-DMA version was emission
bound - 32 indirect_dma_start ops at the Q7 pair-0 rate of ~1.4 us/op feed
gathers at only ~180 GB/s (measured 65 us total). The extended dma_gather op
instead gathers 512 rows per op and is dispatched to Q7 core pair
`queue_num`, so ops round-robined over 4 SWDGE queues emit descriptors on 4
core pairs in parallel. dma_gather requires int16 indices (the Q7 ucode
sign-extends them), so ids must be < 32768: the host compacts the table to
the ~24k rows actually used across the whole batch (uniform 32768 tokens
over 50257 vocab -> E[unique] ~= 24.1k) and remaps ids to int16 local
indices. Every core gets the same compact [32768, 1024] bf16 table. If an
input ever uses > 32768 distinct ids, kernel() falls back to a (slower,
fully general) indirect-DMA module.

dma_gather layout contracts (from the Q7 ucode + CoreSim interpreter):
  - gather position j -> out[j % 128, j // 128, :] (col-major tokens), so
    core token t sits at out[t % 128, t // 128].
  - idx tensor is [128, num_idxs/16] int16: idx j lives at partition j % 16,
    free slot j // 16, replicated across the eight 16-partition groups (each
    Q7 core pair streams its own group).
  - dst must be a [128, n, elem_size] SBUF view; src rows must be contiguous
    with elem bytes % 256 == 0 (bf16 row = 2048 B ok).

Hardware constraints found by probing (CoreSim is more permissive than the
real walrus/NRT stack):
  - walrus requires sync info (a semaphore) on every dynamic DMA, and allows
    at most ONE sync wait attached to a DMA instruction -> raw Block API;
    waits are standalone sequencer instructions.
  - the indirect-DMA offset AP must be [P, 1]; its destination must be a
    whole SBUF tensor at offset 0 (fallback module only).
  - shared-semaphore DMA waits are only unambiguous at full multiples of
    16 * n_ops (SDMA engines complete out of order); per-op semaphores keep
    every wait exact.
"""

import ml_dtypes
import numpy as np

from concourse import bass, library_config, mybir
from concourse.bass_utils import run_bass_kernel_spmd

VOCAB = 50257
D = 1024
B = 8
S = 4096
N_CORES = 8
P = 128
COLS = S // P  # 32 token columns per core

W2_ROWS = 32768  # compact table rows (int16-addressable)
N_QUEUES = 2  # SWDGE queues = Q7 core pairs emitting in parallel
GATHER_OP_TOKENS = 512  # rows per dma_gather op (4 arena columns)


# ---------------------------------------------------------------------------
# fast path: compact table + dma_gather
# ---------------------------------------------------------------------------


def build_module_gather(
    rows=W2_ROWS, d=D, cols=COLS, op_tokens=GATHER_OP_TOKENS, n_queues=N_QUEUES,
    tail_plan=(2, 1, 1),
):
    """SPMD program: int16 local ids -> [P, cols, d] bf16 (col-major tokens)."""
    n_pos = cols * P
    assert n_pos % op_tokens == 0 and op_tokens % P == 0
    ops = []  # (token_lo, token_hi, queue)
    for o, lo in enumerate(range(0, n_pos, op_tokens)):
        ops.append((lo, lo + op_tokens, o % n_queues))
    op_cols = op_tokens // P

    # write chunks in arena columns: one write per gather op, with the last
    # op's columns split finer so the final write after the last gather is
    # one 256KB column
    tail_plan = [w for w in tail_plan if w]
    if sum(tail_plan) != op_cols:
        tail_plan = [1] * op_cols
    w_chunks = []  # (col_lo, col_hi, op_idx)
    for o, (lo, hi, _q) in enumerate(ops):
        clo, chi = lo // P, hi // P
        if o == len(ops) - 1:
            for w in tail_plan:
                w_chunks.append((clo, clo + w, o))
                clo += w
        else:
            w_chunks.append((clo, chi, o))

    nc = bass.Bass(
        "TRN2",
        enable_partition_id=False,
        detect_race_conditions=False,
        num_swdge_queues=n_queues,
    )
    tok = nc.declare_dram_parameter(
        "token_ids", [P, n_pos // 16], mybir.dt.int16, isOutput=False
    )
    w = nc.declare_dram_parameter("weight", [rows, d], mybir.dt.bfloat16, isOutput=False)
    out = nc.declare_dram_parameter("out", [P, cols, d], mybir.dt.bfloat16, isOutput=True)

    with (
        nc.Block(no_gpsimd_drain=True) as block,
        nc.semaphore("idx_sem") as idx_sem,
        nc.semaphore("w_sem") as w_sem,
    ):
        idx = nc.alloc_sbuf_tensor("idx", [P, n_pos // 16], mybir.dt.int16)
        arena = nc.alloc_sbuf_tensor("arena", [P, cols, d], mybir.dt.bfloat16)
        g_sems = [nc.semaphore(f"g_sem{o}").__enter__() for o in range(len(ops))]

        @block.gpsimd
        def _(g: bass.BassGpSimd):
            # dma_gather lives in the 'mlp' GPSIMD library; the IRAM reload
            # overlaps the idx-load DMA
            g.load_library(library_config.mlp)
            g.wait_ge(idx_sem, 16)
            for o, (lo, hi, q) in enumerate(ops):
                g.dma_gather(
                    arena[:, lo // P : hi // P, :],
                    w[:],
                    idx[:, lo // 16 : hi // 16],
                    num_idxs=hi - lo,
                    num_idxs_reg=hi - lo,
                    elem_size=d,
                    queue_num=q,
                    # one packet per 2KB descriptor: with the default
                    # whole-op coalescing each SDMA engine drains the full
                    # 64KB op share before servicing the HWDGE write queue,
                    # serializing the gather and write phases
                    single_packet=False,
                ).then_inc(g_sems[o], 16)

        # writes alternate between the two HWDGE rings (sync + scalar
        # engines) so the write stream holds its own in the SDMA engines'
        # per-ring round-robin against the gather rings
        sync_chunks = w_chunks[0::2]
        scalar_chunks = w_chunks[1::2]

        @block.scalar
        def _(a: bass.BassEngine):
            for clo, chi, o in scalar_chunks:
                a.wait_ge(g_sems[o], 16)
                a.dma_start(
                    out=out[:, clo:chi, :], in_=arena[:, clo:chi, :]
                ).then_inc(w_sem, 16)

        @block.sync
        def _(s: bass.BassEngine):
            s.dma_start(out=idx[:], in_=tok[:]).then_inc(idx_sem, 16)
            for clo, chi, o in sync_chunks:
                s.wait_ge(g_sems[o], 16)
                s.dma_start(
                    out=out[:, clo:chi, :], in_=arena[:, clo:chi, :]
                ).then_inc(w_sem, 16)
            s.wait_ge(w_sem, 16 * len(w_chunks))

    # raw Bass skips Bacc's codegen_inst_isa_subclasses pass; without it the
    # NEFF compiler sees empty .instr on extended insts -> "ISA wrong length"
    mybir.codegen_inst_isa_subclasses(nc)
    return nc


# ---------------------------------------------------------------------------
# fallback path: replicated full table + indirect DMA (any vocab size)
# ---------------------------------------------------------------------------

CHUNK_PLAN = [1] * 32


def build_module_indirect(vocab=VOCAB, d=D, cols=COLS, chunk_plan=None, sim_mode=False):
    """SPMD program: [P, cols] int32 token ids -> [P, cols, d] bf16.

    sim_mode=True: per-column output writes read the per-column alias tiles
    instead of the contiguous arena (CoreSim's tensor-granular race checker
    flags the aliased arena read; hardware structure is identical).

    Write chunks are single columns (2KB-per-partition descriptors): with
    wider chunks (4KB descs) the SDMA per-packet round-robin gives the write
    queue a larger byte share than the gather queue, so gather data falls
    ~1MB behind its (rate-limiting) Q7 emission and the kernel pays a ~5.6us
    drain at the end.
    """
    if chunk_plan is None:
        chunk_plan = [1] * cols
    assert sum(chunk_plan) == cols, chunk_plan
    chunks = []
    lo = 0
    for w in chunk_plan:
        chunks.append((lo, lo + w))
        lo += w

    nc = bass.Bass("TRN2", enable_partition_id=False, detect_race_conditions=False)
    tok = nc.declare_dram_parameter("token_ids", [P, cols], mybir.dt.int32, isOutput=False)
    w = nc.declare_dram_parameter("weight", [vocab, d], mybir.dt.bfloat16, isOutput=False)
    out = nc.declare_dram_parameter("out", [P, cols, d], mybir.dt.bfloat16, isOutput=True)

    row_bytes = d * 2

    with (
        nc.Block(no_gpsimd_drain=True) as block,
        nc.semaphore("idx_sem") as idx_sem,
        nc.semaphore("w_sem") as w_sem,
    ):
        idx = nc.alloc_sbuf_tensor("idx", [P, cols], mybir.dt.int32)
        gbig = nc.alloc_sbuf_tensor("gbig", [P, cols * d], mybir.dt.bfloat16)
        base = nc.lookup_mloc(gbig).addr
        tiles = [
            nc.alloc_sbuf_tensor_at(
                f"ga{c}", [P, d], mybir.dt.bfloat16, offset=base + c * row_bytes
            )
            for c in range(cols)
        ]
        c_sems = [nc.semaphore(f"c_sem{k}").__enter__() for k in range(len(chunks))]

        @block.gpsimd
        def _(g: bass.BassEngine):
            g.wait_ge(idx_sem, 16)
            for k, (lo, hi) in enumerate(chunks):
                for c in range(lo, hi):
                    g.indirect_dma_start(
                        out=tiles[c][:],
                        out_offset=None,
                        in_=w[:],
                        in_offset=bass.IndirectOffsetOnAxis(
                            ap=idx[:, c : c + 1], axis=0
                        ),
                    ).then_inc(c_sems[k], 16)

        @block.sync
        def _(s: bass.BassEngine):
            s.dma_start(out=idx[:], in_=tok[:]).then_inc(idx_sem, 16)
            for k, (lo, hi) in enumerate(chunks):
                s.wait_ge(c_sems[k], 16 * (hi - lo))
                if sim_mode:
                    s.dma_start(
                        out=out[:, lo:hi, :], in_=tiles[lo][:]
                    ).then_inc(w_sem, 16)
                else:
                    s.dma_start(
                        out=out[:, lo:hi, :], in_=gbig[:, lo * d : hi * d]
                    ).then_inc(w_sem, 16)
            s.wait_ge(w_sem, 16 * len(chunks))

    return nc


_module_cache = {}


def _get_module(kind):
    if kind not in _module_cache:
        _module_cache[kind] = (
            build_module_gather() if kind == "gather" else build_module_indirect()
        )
    return _module_cache[kind]


def _pack_idx16(lids):
    """[S] int16 gather-position ids -> [P, S/16] SBUF idx layout: idx j at
    partition j % 16, free slot j // 16, replicated across the 8 groups."""
    x = np.ascontiguousarray(lids.reshape(-1, 16).T)  # [16, S/16]
    return np.tile(x, (P // 16, 1))


def kernel(token_ids, weight, **run_kwargs):
    # The indirect-DMA module wins on hardware: the dma_gather path's
    # mandatory GPSIMD library load (~9.5 us of dead start-up on the Q7
    # stream) exceeds its faster-emission advantage (~8 us) at this size.
    token_ids = np.asarray(token_ids)
    weight = np.asarray(weight, dtype=np.float32)
    assert token_ids.shape == (B, S), token_ids.shape
    assert weight.shape == (VOCAB, D), weight.shape
    w_bf16 = weight.astype(ml_dtypes.bfloat16)

    nc = _get_module("indirect")
    ids32 = np.ascontiguousarray(token_ids.astype(np.int32))
    in_maps = [
        {"token_ids": ids32[i].reshape(P, COLS), "weight": w_bf16}
        for i in range(N_CORES)
    ]
    res = run_bass_kernel_spmd(
        nc, in_maps, core_ids=list(range(N_CORES)), **run_kwargs
    )
    # out[p, c] = token p*COLS + c -> plain reshape
    out = np.stack(
        [np.asarray(res.results[i]["out"]).reshape(S, D) for i in range(N_CORES)]
    ).astype(np.float32).reshape(B, S, D)
    if run_kwargs:
        return out, res
    return out
